# revision 13
# baseline (speedup 1.0000x reference)
"""Deformable Conv2D (DCNv2 forward) — v7b: transposed G + elem_step x-pair gather.

Changes vs v6 baseline:
- GEMM computes G_k^T directly (lhsT=img, rhs=filt -> PSUM [pix, OH]):
  no xbar transposes at all.
- UNduplicated tables gda=[taps 0..5], gdb=[taps 6..8] of [PIX, OH] bf16 rows
  (18.9MB/iter written instead of 37.7MB), written contiguously.
- dma_gather elem_step (256B) < elem_size (512B): one 512B descriptor covers
  the bilinear x-pair (table rows s, s+1) WITHOUT any table duplication.
  The y-pair is two tokens (idx and idx+64). 73728 tokens/iter of 512B.
- Combine is position-major [p=pos, k, t, y, x, o]: weight multiply is a
  per-partition broadcast TT (w [p,k,t,y,x] over o), then contiguous bf16
  pairwise adds (DVE 2x/4x fast modes) replace the strided TensorReduces.
- Tables double-buffered by iteration parity so Phase G(i+1) overlaps
  Phase S(i).
"""

import sys

sys.path.insert(0, "/opt/trn_rl_repo")

import numpy as np
from contextlib import ExitStack

import concourse.bass as bass
import concourse.mybir as mybir
import concourse.tile as tile
from concourse import bacc
from concourse import bass_utils
from concourse.ap import AP

P = 128
H = W = 64
C = 256
CO = 256
K = 9
PIX = H * W
NT = PIX // P        # 32 position tiles
OH = 128             # cout half per core
KA, KB = 6, 3        # tap split for int16 gather indices
TCH = 4              # position tiles per gather chunk
NCH = NT // TCH      # 8 chunks
NIA = KA * TCH * 2 * P   # idxs per chunk, table A (6144)
NIB = KB * TCH * 2 * P   # idxs per chunk, table B (3072)
IDXW = (NIA + NIB) // 16  # i16 per partition per chunk (576)
F32 = mybir.dt.float32
BF16 = mybir.dt.bfloat16
I16 = mybir.dt.int16
AL = mybir.AluOpType
AF = mybir.ActivationFunctionType

_PROGRAM_CACHE = {}


def _build_program(iters=1):
    key = ("v7b", iters)
    if key in _PROGRAM_CACHE:
        return _PROGRAM_CACHE[key]

    nc = bacc.Bacc(
        "TRN2",
        target_bir_lowering=False,
        debug=False,
        enable_asserts=False,
        num_devices=8,
    )
    img_d = nc.dram_tensor("img", [P, 2 * PIX], BF16, kind="ExternalInput")
    filt_d = nc.dram_tensor("filt", [P, 2 * K * OH], BF16, kind="ExternalInput")
    # per-position weights, 8x-expanded: [p, chunk, k, t_local, y, x, 8] bf16
    wt_d = nc.dram_tensor("wt", [P, K * NT * 4 * 8], BF16,
                          kind="ExternalInput")
    gidx_d = nc.dram_tensor("gidx", [P, NCH * IDXW], I16, kind="ExternalInput")
    out_d = nc.dram_tensor("outp", [P, NT * OH], F32, kind="ExternalOutput")
    # G^T tables (rows = tap-local pixel), double-buffered by parity
    gda = [nc.dram_tensor(f"gda{par}", [KA * PIX, OH], BF16, kind="Internal")
           for par in range(2)]
    gdb = [nc.dram_tensor(f"gdb{par}", [KB * PIX, OH], BF16, kind="Internal")
           for par in range(2)]

    with tile.TileContext(nc) as tc, ExitStack() as ctx:
        cp = ctx.enter_context(tc.tile_pool(name="const", bufs=1))
        pg = ctx.enter_context(tc.tile_pool(name="gsb", bufs=2))
        pga = ctx.enter_context(tc.tile_pool(name="gath", bufs=2))
        pa = ctx.enter_context(tc.tile_pool(name="acc", bufs=2))
        pps = ctx.enter_context(tc.tile_pool(name="ps", bufs=6, space="PSUM"))

        filt_sb = cp.tile([P, 2, K, OH], BF16, name="filt_sb")
        nc.sync.dma_start(
            filt_sb[:], filt_d.ap().rearrange("p (c k o) -> p c k o", c=2, k=K)
        )
        wt_sb = cp.tile([P, NCH, K * TCH * 4 * 8], BF16, name="wt_sb")
        nc.sync.dma_start(
            wt_sb[:], wt_d.ap().rearrange(
                "p (c w) -> p c w", c=NCH))
        gidx_sb = cp.tile([P, NCH, IDXW], I16, name="gidx_sb")
        nc.sync.dma_start(
            gidx_sb[:], gidx_d.ap().rearrange("p (c w) -> p c w", c=NCH))
        img_bf = cp.tile([P, 2, PIX], BF16, name="img_bf")
        nc.sync.dma_start(
            img_bf[:], img_d.ap().rearrange("p (c x) -> p c x", c=2))

        for it in range(iters):
            par = it % 2
            # ---- Phase G: G_k^T = img^T @ filt -> DRAM tables ----
            for g in range(3):            # tap groups of 3
                for th in range(2):       # halves of the 32 pix-tiles
                    gsb = pg.tile([P, NT // 2, 3, OH], BF16,
                                  name="gsb", tag="gsb")
                    for t in range(NT // 2):
                        ps = pps.tile([P, 3 * OH], F32, name="ps", tag="ps")
                        pix0 = (th * (NT // 2) + t) * P
                        for cc in range(2):
                            nc.tensor.matmul(
                                ps[:],
                                lhsT=img_bf[:, cc, pix0:pix0 + P],
                                rhs=filt_sb[:, cc, 3 * g:3 * g + 3, :]
                                .rearrange("p k o -> p (k o)"),
                                start=(cc == 0),
                                stop=(cc == 1),
                            )
                        nc.scalar.activation(
                            gsb[:, t].rearrange("p a b -> p (a b)"),
                            ps[:], AF.Copy)
                    for kk in range(3):
                        k = 3 * g + kk
                        if k < KA:
                            dst = gda[par].ap()[k * PIX:(k + 1) * PIX]
                        else:
                            dst = gdb[par].ap()[
                                (k - KA) * PIX:(k - KA + 1) * PIX]
                        dstv = dst.rearrange("(t p) o -> p t o", p=P)
                        nc.sync.dma_start(
                            dstv[:, th * (NT // 2):(th + 1) * (NT // 2), :],
                            gsb[:, :, kk, :])

            # ---- Phase S: gather + weighted combine ----
            for c in range(NCH):
                ts = slice(c * TCH, (c + 1) * TCH)
                # slots: (k, t_local, y); A = slots [0 : KA*TCH*2)
                gth = pga.tile([P, K * TCH * 2, 2, OH], BF16,
                               name="gth", tag="gth")
                srcA = AP(gda[par], 0, [[OH, KA * PIX - 1], [1, 2 * OH]])
                srcB = AP(gdb[par], 0, [[OH, KB * PIX - 1], [1, 2 * OH]])
                nc.gpsimd.dma_gather(
                    out_ap=gth[:, 0:KA * TCH * 2]
                    .rearrange("p j x o -> p j (x o)"),
                    in_ap=srcA,
                    idxs_ap=gidx_sb[:, c, 0:NIA // 16],
                    num_idxs=NIA,
                    num_idxs_reg=NIA,
                    elem_size=2 * OH,
                    elem_step=OH,
                    single_packet=False,
                )
                nc.gpsimd.dma_gather(
                    out_ap=gth[:, KA * TCH * 2:]
                    .rearrange("p j x o -> p j (x o)"),
                    in_ap=srcB,
                    idxs_ap=gidx_sb[:, c, NIA // 16:],
                    num_idxs=NIB,
                    num_idxs_reg=NIB,
                    elem_size=2 * OH,
                    elem_step=OH,
                    single_packet=False,
                )
                # gth [p, (k t y), x, o]; weighted combine all in-place.
                # mult at 4x: weights 8x-expanded, broadcast on MIDDLE dim.
                gv16 = gth[:].rearrange("p j x (ob e) -> p (j x) ob e", e=8)
                w8 = wt_sb[:, c].rearrange(
                    "p (j e) -> p j e", e=8)[:, :, None, :].to_broadcast(
                    (P, K * TCH * 4, OH // 8, 8))
                nc.vector.tensor_tensor(gv16, gv16, w8, AL.mult)
                # views for the in-place add tree
                g6 = gth[:].rearrange("p (k t y) x o -> p (k t) y x o",
                                      k=K, t=TCH)
                tx0 = g6[:, :, :, 0, :]    # x-sum -> x0 slots
                nc.vector.tensor_tensor(
                    tx0, tx0, g6[:, :, :, 1, :], AL.add)
                ty0 = g6[:, :, 0, 0, :]    # y-sum -> y0/x0 slots [p, kt, o]
                nc.vector.tensor_tensor(
                    ty0, ty0, g6[:, :, 1, 0, :], AL.add)
                # tap tree over k on Pool (small tail ops): [p, k, t, o]
                tk = gth[:].rearrange("p (k t y) x o -> p k t y x o",
                                      k=K, t=TCH)[:, :, :, 0, 0, :]
                nc.gpsimd.tensor_tensor(
                    tk[:, 0:4], tk[:, 0:4], tk[:, 4:8], AL.add)
                nc.gpsimd.tensor_tensor(
                    tk[:, 0:2], tk[:, 0:2], tk[:, 2:4], AL.add)
                nc.gpsimd.tensor_tensor(
                    tk[:, 0], tk[:, 0], tk[:, 1], AL.add)
                acc = pa.tile([P, TCH, OH], F32, name="acc", tag="acc")
                nc.gpsimd.tensor_tensor(
                    acc[:], tk[:, 0], tk[:, 8], AL.add)
                nc.sync.dma_start(
                    out_d.ap().rearrange("p (t o) -> p t o", t=NT)[:, ts, :],
                    acc[:])

    nc.compile()
    from concourse.bass_interp import get_hw_module

    nc.m = get_hw_module(nc.m)
    _PROGRAM_CACHE[key] = nc
    return nc


def _pack_filt(filt, j):
    import ml_dtypes
    Wm = filt.reshape(CO, C, K)[j * OH:(j + 1) * OH]
    T = Wm.transpose(1, 2, 0).reshape(2, P, K, OH)
    return np.ascontiguousarray(
        T.transpose(1, 0, 2, 3).reshape(P, 2 * K * OH)
    ).astype(ml_dtypes.bfloat16)


def _host_sample_math(offset_n, mask_n):
    """Returns Wc [K,2x,2y,PIX] f32 weights and rows [K,PIX] i64 (y0 rows)."""
    pos_y = (np.arange(PIX) // W).astype(np.float64)
    pos_x = (np.arange(PIX) % W).astype(np.float64)
    Wc = np.empty((K, 2, 2, PIX), np.float32)
    rows = np.empty((K, PIX), np.int64)
    for k in range(K):
        ki, kj = k // 3, k % 3
        y = pos_y - 1.0 + ki + offset_n[2 * k].ravel().astype(np.float64)
        x = pos_x - 1.0 + kj + offset_n[2 * k + 1].ravel().astype(np.float64)
        m = mask_n[k].ravel().astype(np.float64)
        ws = []
        ss = []
        for v in (y, x):
            f = np.floor(v)
            l = v - f
            s = np.clip(f, 0, 62)
            u0 = (1.0 - l) * (f == np.clip(f, 0, 63))
            u1 = l * ((f + 1) == np.clip(f + 1, 0, 63))
            w_s0 = u0 * (f == s) + u1 * ((f + 1) == s)
            w_s1 = u1 * ((f + 1) == (s + 1)) + u0 * (f == (s + 1))
            ws.append((w_s0, w_s1))
            ss.append(s.astype(np.int64))
        ys, xs = ss
        rows[k] = ys * W + xs
        for xa in range(2):
            for ya in range(2):
                Wc[k, xa, ya] = (ws[0][ya] * ws[1][xa] * m).astype(np.float32)
    return Wc, rows


def _pack_wt(Wc):
    # Wc [K, 2x, 2y, PIX] -> [p, (chunk, k, t_local, y, x, e8)] bf16
    import ml_dtypes
    A = Wc.reshape(K, 2, 2, NCH, TCH, P)    # [k, x, y, c, tl, p]
    A = A.transpose(5, 3, 0, 4, 2, 1)       # [p, c, k, tl, y, x]
    A = np.repeat(A[..., None], 8, axis=-1)
    return np.ascontiguousarray(
        A.reshape(P, K * NT * 4 * 8)).astype(ml_dtypes.bfloat16)


def _pack_gidx(rows):
    # rows [K, PIX] (y0 rows, tap-local) -> [128, NCH*IDXW] i16 wrapped.
    # chunk c tokens: slot j = (k, t_local, y), token i = j*128 + p.
    # idx value = k_local*PIX + rows + y*W into table A (k<KA) or B.
    out = np.empty((16, NCH, IDXW), np.int16)
    r = rows.reshape(K, NT, P)            # [k, t, p]
    for c in range(NCH):
        rc = r[:, c * TCH:(c + 1) * TCH, :]          # [k, t, p]
        idx = np.empty((K, TCH, 2, P), np.int64)
        for k in range(K):
            base = (k if k < KA else k - KA) * PIX
            idx[k, :, 0, :] = base + rc[k]
            idx[k, :, 1, :] = base + rc[k] + W
        blkA = idx[:KA].reshape(NIA)
        blkB = idx[KA:].reshape(NIB)
        assert blkA.max() < KA * PIX - 1 and blkB.max() < KB * PIX - 1
        assert blkA.min() >= 0 and blkB.min() >= 0
        out[:, c, :NIA // 16] = blkA.astype(np.int16).reshape(NIA // 16, 16).T
        out[:, c, NIA // 16:] = blkB.astype(np.int16).reshape(NIB // 16, 16).T
    out = out.reshape(16, NCH * IDXW)
    return np.ascontiguousarray(out[np.arange(P) % 16])


def make_in_maps(inputs, filter, offset, mask):
    inputs = np.ascontiguousarray(np.asarray(inputs, np.float32))
    filter = np.ascontiguousarray(np.asarray(filter, np.float32))
    offset = np.ascontiguousarray(np.asarray(offset, np.float32))
    mask = np.ascontiguousarray(np.asarray(mask, np.float32))
    filt_j = [_pack_filt(filter, j) for j in range(2)]
    per_n = []
    for n in range(4):
        Wc, rows = _host_sample_math(offset[n], mask[n])
        im = inputs[n].reshape(2, P, PIX).transpose(1, 0, 2).reshape(P, 2 * PIX)
        import ml_dtypes
        im = np.ascontiguousarray(im).astype(ml_dtypes.bfloat16)
        per_n.append((im, _pack_wt(Wc), _pack_gidx(rows)))
    in_maps = []
    for core in range(8):
        n, j = core // 2, core % 2
        im, wt, gidx = per_n[n]
        in_maps.append({"img": im, "filt": filt_j[j], "wt": wt, "gidx": gidx})
    return in_maps


def assemble_output(results):
    out = np.zeros((4, CO, H, W), np.float32)
    for core in range(8):
        n, j = core // 2, core % 2
        r = np.asarray(results[core]["outp"])      # [p, t*o]
        r = r.reshape(P, NT, OH).transpose(2, 1, 0)  # [o, t, p]
        out[n][j * OH:(j + 1) * OH] = r.reshape(OH, H, W)
    return out


_RUNNER_CACHE = {}


def _fast_run(nc, in_maps):
    """Persistent-jit dispatch (axon): avoids per-call retrace/recompile."""
    import jax
    from jax.sharding import Mesh, PartitionSpec
    from jax.experimental.shard_map import shard_map
    from concourse.bass2jax import (
        _bass_exec_p, install_neuronx_cc_hook, partition_id_tensor)

    key = id(nc)
    if key not in _RUNNER_CACHE:
        install_neuronx_cc_hook()
        pname = nc.partition_id_tensor.name if nc.partition_id_tensor else None
        in_names, out_names, out_avals = [], [], []
        for alloc in nc.m.functions[0].allocations:
            if not isinstance(alloc, mybir.MemoryLocationSet):
                continue
            name = alloc.memorylocations[0].name
            if alloc.kind == "ExternalInput":
                if name != pname:
                    in_names.append(name)
            elif alloc.kind == "ExternalOutput":
                out_names.append(name)
                out_avals.append(jax.core.ShapedArray(
                    tuple(alloc.tensor_shape), mybir.dt.np(alloc.dtype)))
        all_in = list(in_names) + out_names + ([pname] if pname else [])

        def _body(*args):
            operands = list(args)
            if pname is not None:
                operands.append(partition_id_tensor())
            return tuple(_bass_exec_p.bind(
                *operands, out_avals=tuple(out_avals),
                in_names=tuple(all_in), out_names=tuple(out_names),
                lowering_input_output_aliases=(),
                sim_require_finite=True, sim_require_nnan=True, nc=nc))

        devices = jax.devices()[:8]
        mesh = Mesh(np.asarray(devices), ("core",))
        nio = len(in_names) + len(out_names)
        fn = jax.jit(shard_map(
            _body, mesh=mesh, in_specs=(PartitionSpec("core"),) * nio,
            out_specs=(PartitionSpec("core"),) * len(out_names),
            check_rep=False), keep_unused=True)
        _RUNNER_CACHE[key] = (fn, in_names, out_names, out_avals)
    fn, in_names, out_names, out_avals = _RUNNER_CACHE[key]
    concat_in = [np.concatenate([np.asarray(m[nm]) for m in in_maps], axis=0)
                 for nm in in_names]
    concat_zero = [np.zeros((8 * a.shape[0], *a.shape[1:]), a.dtype)
                   for a in out_avals]
    outs = fn(*concat_in, *concat_zero)
    return [
        {nm: np.asarray(outs[i]).reshape(8, *out_avals[i].shape)[c]
         for i, nm in enumerate(out_names)}
        for c in range(8)
    ]


def kernel(inputs, filter, offset, mask):
    nc = _build_program()
    in_maps = make_in_maps(inputs, filter, offset, mask)
    try:
        if bass_utils.axon_active():
            results = _fast_run(nc, in_maps)
        else:
            results = bass_utils.run_bass_kernel_spmd(
                nc, in_maps, core_ids=list(range(8))).results
    except Exception:
        results = bass_utils.run_bass_kernel_spmd(
            nc, in_maps, core_ids=list(range(8))).results
    return assemble_output(results)
